# revision 2
# baseline (speedup 1.0000x reference)
"""Trainium2 Bass kernel for nn_Classifier_8418135900320 (retrieval_knn).

Reference computes, for S[i,j] = cos(y_i, z_j):
  top1  = mean_i(argmax_j S[i,j] == i)
  top10 = mean_i(i in top-10 indices of row i)

Both reduce to per-row counting: with cnt[i] = #{j : S[i,j] > S[i,i]},
  top1  = mean(cnt == 0),  top10 = mean(cnt <= 9).

Row-scaling by 1/||y_i|| never changes per-row comparisons, so we only
normalize Z on the host (W = Z/||z_j||, cast fp16) and compute
G[i,j] = y_i . w_j on device.  A single fp16 matmul pass has dot-product
error ~2e-4 while the minimum decision margin on this data distribution is
~1e-2 -- orders of magnitude of safety.

Sharding: rows of Y (queries) across 8 cores, W replicated.  W is rotated
by -1024*c rows for core c so the diagonal entries of the local [1024,8192]
score slab sit at a fixed position (col == local row) on every core,
letting all cores run one SPMD program.

Per core: 8 row-tiles x 16 col-tiles of [128,512] PSUM scores (4 fp16
matmuls each, K=128 chunks).  The diagonal value is extracted from the
same PSUM values (copy slice -> mask diag -> reduce), so the strict
is_gt comparison is exactly self-excluding.  Compare+count runs as fused
compare+accumulate ops, split between the Vector engine (is_gt) and the
Scalar engine (Sign with bias=-diag) to balance load.  Per-row counts and
per-core accuracy partial sums are DMA'd out; the host sums 8 pairs.
"""

import numpy as np

B = 8192
D = 512
NCORES = 8
BL = B // NCORES  # 1024 local rows per core
P = 128           # partitions
KC = D // P       # 4 contraction chunks
RT = BL // P      # 8 row tiles
NW = 512          # matmul moving free dim / PSUM bank width (fp32)
CTN = B // NW     # 16 col tiles

_compiled = None


def _build_program():
    import concourse.bass as bass
    import concourse.bacc as bacc
    import concourse.tile as tile
    from concourse import mybir

    f32 = mybir.dt.float32
    f16 = mybir.dt.float16
    bf16 = mybir.dt.bfloat16
    AL = mybir.AluOpType
    AF = mybir.ActivationFunctionType
    AX = mybir.AxisListType

    nc = bacc.Bacc("TRN2", target_bir_lowering=False, num_devices=NCORES)

    yt = nc.declare_dram_parameter("yt", [D, BL], f16, isOutput=False)
    wt = nc.declare_dram_parameter("wt", [D, B], f16, isOutput=False)
    cnt_d = nc.declare_dram_parameter("cnt", [RT, P], f32, isOutput=True)
    acc_d = nc.declare_dram_parameter("acc", [1, 2 * RT], f32, isOutput=True)

    with tile.TileContext(nc) as tc:
        with (
            tc.tile_pool(name="wpool", bufs=1) as wpool,
            tc.tile_pool(name="ypool", bufs=1) as ypool,
            tc.tile_pool(name="psum", bufs=6, space=bass.MemorySpace.PSUM) as pspool,
            tc.tile_pool(name="accps", bufs=1, space=bass.MemorySpace.PSUM) as accpool,
            tc.tile_pool(name="daux", bufs=2) as daux,
            tc.tile_pool(name="scr", bufs=3) as scrpool,
            tc.tile_pool(name="percol", bufs=RT) as percol,
            tc.tile_pool(name="redu", bufs=2) as redu,
            tc.tile_pool(name="persist", bufs=1) as persist,
        ):
            w16 = wpool.tile([P, KC, B], f16)
            y16 = ypool.tile([P, KC, BL], f16)

            # Loads: y first (small), then W in column strips so early
            # col-tiles unblock while later strips stream in.
            for k in range(KC):
                nc.gpsimd.dma_start(y16[:, k, :], yt[k * P:(k + 1) * P, :])
            SW = 2048
            for s in range(B // SW):
                for k in range(KC):
                    nc.gpsimd.dma_start(
                        w16[:, k, s * SW:(s + 1) * SW],
                        wt[k * P:(k + 1) * P, s * SW:(s + 1) * SW],
                    )

            ones = persist.tile([P, 1], bf16)
            nc.gpsimd.memset(ones[:], 1.0)
            cntsb = persist.tile([P, RT], f32)
            flags = persist.tile([P, 2, RT], bf16)

            dp = {}
            dpn = {}
            cd = {}
            sa = {}
            pick = {}
            n_dve = {}
            n_act = {}

            def start_rt(rt):
                pick[rt] = 0
                n_dve[rt] = 0
                n_act[rt] = 0
                cd[rt] = percol.tile([P, CTN], f32, tag="cd", name=f"cd{rt}")
                sa[rt] = percol.tile([P, CTN], f32, tag="sa", name=f"sa{rt}")
                dp[rt] = percol.tile([P, 1], f32, tag="dp", name=f"dp{rt}")
                dpn[rt] = percol.tile([P, 1], f32, tag="dpn", name=f"dpn{rt}")

            def emit_tile(rt, ct):
                ct_d = (rt * P) // NW
                off = (rt * P) % NW
                pt = pspool.tile([P, NW], f32, tag="pt")
                for k in range(KC):
                    nc.tensor.matmul(
                        pt[:],
                        y16[:, k, rt * P:(rt + 1) * P],
                        w16[:, k, ct * NW:(ct + 1) * NW],
                        start=(k == 0),
                        stop=(k == KC - 1),
                    )
                if ct == ct_d:
                    # Extract diag from the same PSUM values: exact
                    # self-exclusion under strict is_gt.
                    dblk = daux.tile([P, P], f32, tag="dblk")
                    nc.scalar.copy(dblk[:], pt[:, off:off + P])
                    dmask = daux.tile([P, P], f32, tag="dmask")
                    nc.gpsimd.affine_select(
                        dmask[:],
                        dblk[:],
                        pattern=[[-1, P]],
                        compare_op=AL.is_equal,
                        fill=0.0,
                        base=0,
                        channel_multiplier=1,
                    )
                    nc.vector.tensor_reduce(dp[rt][:], dmask[:], AX.X, AL.add)
                    nc.vector.tensor_scalar_mul(dpn[rt][:], dp[rt][:], -1.0)
                use_dve = (ct == ct_d) or (pick[rt] % 2 == 1)
                pick[rt] += 1
                if use_dve:
                    scr = scrpool.tile([P, NW], bf16, tag="scr_dve")
                    i = n_dve[rt]
                    n_dve[rt] += 1
                    nc.vector.tensor_scalar(
                        scr[:],
                        pt[:],
                        dp[rt][:],
                        None,
                        op0=AL.is_gt,
                        op1=AL.add,
                        accum_out=cd[rt][:, i:i + 1],
                    )
                else:
                    scr = scrpool.tile([P, NW], bf16, tag="scr_act")
                    i = n_act[rt]
                    n_act[rt] += 1
                    nc.scalar.activation(
                        scr[:],
                        pt[:],
                        AF.Sign,
                        bias=dpn[rt][:],
                        scale=1.0,
                        accum_out=sa[rt][:, i:i + 1],
                    )

            def finish_rt(rt):
                # cnt = sum(dve counts) + (sum(act sign-sums) + 512*n_act)/2
                c1 = redu.tile([P, 1], f32, tag="c1")
                nc.vector.tensor_reduce(c1[:], cd[rt][:, :n_dve[rt]], AX.X, AL.add)
                s1 = redu.tile([P, 1], f32, tag="s1")
                nc.vector.tensor_reduce(s1[:], sa[rt][:, :n_act[rt]], AX.X, AL.add)
                s2 = redu.tile([P, 1], f32, tag="s2")
                nc.vector.tensor_scalar(
                    s2[:], s1[:], 0.5, 256.0 * n_act[rt], op0=AL.mult, op1=AL.add
                )
                nc.vector.tensor_add(cntsb[:, rt:rt + 1], c1[:], s2[:])
                nc.vector.tensor_scalar(
                    flags[:, 0, rt:rt + 1], cntsb[:, rt:rt + 1], 0.5, None,
                    op0=AL.is_lt, op1=AL.bypass,
                )
                nc.vector.tensor_scalar(
                    flags[:, 1, rt:rt + 1], cntsb[:, rt:rt + 1], 9.5, None,
                    op0=AL.is_lt, op1=AL.bypass,
                )
                nc.gpsimd.dma_start(cnt_d[rt, :], cntsb[:, rt:rt + 1])

            # Order: diag tiles first per row-tile so dp is ready before the
            # bulk of the compares; ct ascending afterwards matches the DMA
            # strip arrival order.
            order = []
            for rt in range(4):
                order.append((rt, 0))      # diag tiles of rt 0..3
            for rt in range(4, 8):
                order.append((rt, 1))      # diag tiles of rt 4..7
            for rt in range(4):
                order.append((rt, 1))
            for rt in range(4, 8):
                order.append((rt, 0))
            for ct in range(2, CTN):
                for rt in range(RT):
                    order.append((rt, ct))

            for rt in range(RT):
                start_rt(rt)
            done = {rt: 0 for rt in range(RT)}
            for rt, ct in order:
                emit_tile(rt, ct)
                done[rt] += 1
                if done[rt] == CTN:
                    finish_rt(rt)

            # Per-core accuracy partial sums: column-sums of flags via a
            # ones-vector matmul (partition reduction on the PE).
            accps = accpool.tile([1, 2 * RT], f32)
            nc.tensor.matmul(accps[:], ones[:], flags[:, :, :], start=True, stop=True)
            accsb = persist.tile([1, 2 * RT], f32)
            nc.scalar.copy(accsb[:], accps[:])
            nc.gpsimd.dma_start(acc_d[:], accsb[:])

    nc.compile()
    return nc


def _prep_inputs(Z, Y):
    Z = np.asarray(Z, dtype=np.float32)
    Y = np.asarray(Y, dtype=np.float32)
    zn = np.sqrt((Z.astype(np.float64) ** 2).sum(axis=1))
    W16 = (Z.astype(np.float64) / zn[:, None]).astype(np.float16)
    Y16 = Y.astype(np.float16)
    in_maps = []
    for c in range(NCORES):
        Wc = np.roll(W16, -BL * c, axis=0)
        in_maps.append({
            "wt": np.ascontiguousarray(Wc.T),
            "yt": np.ascontiguousarray(Y16[c * BL:(c + 1) * BL].T),
        })
    return in_maps


def _run(in_maps, trace=False):
    global _compiled
    if _compiled is None:
        _compiled = _build_program()
    from concourse.bass_utils import run_bass_kernel_spmd
    return run_bass_kernel_spmd(_compiled, in_maps, list(range(NCORES)), trace=trace)


def kernel(Z, Y):
    in_maps = _prep_inputs(Z, Y)
    res = _run(in_maps)
    acc = np.zeros(2, dtype=np.float64)
    for c in range(NCORES):
        a = np.asarray(res.results[c]["acc"], dtype=np.float64).reshape(2, RT)
        acc += a.sum(axis=1)
    top1 = np.float32(acc[0] / B)
    top10 = np.float32(acc[1] / B)
    return (top1, top10)
